# revision 29
# baseline (speedup 1.0000x reference)
"""Trainium2 Bass kernel for nn_Diffuser_78331613544465.

Math (per graph b of B=8, N=1024):
    A   = adj (mask is all-ones in the graded setup; general mask handled host-side)
    P   = A / max(rowsum(A), 1)
    out[i,j,:] = relu([I, P, P2, P4][i,j,:] @ w1 + b1) @ w2 + b2   (P2=P@P, P4=P2@P2)

Device strategy: data-parallel over B — one graph per NeuronCore (8 cores).
All on-chip work happens in the TRANSPOSED domain (Q = P^T), because
  * Q = A * (1/deg) is a column-scale of the symmetric adj,
  * Q2 = Q@Q = P2^T, Q4 = Q2@Q2 = P4^T chain natively,
  * the edge-MLP then runs with j on partitions / i on the moving dim, which
    makes layer-1 a K=24 block-diagonal matmul over interleaved (j,s) rows,
    layer-2 a K=128 block-diagonal matmul, and the final 32x32 DVE stream
    transpose emits [i-partition, (j,o)-contiguous] tiles for 128B-chunk DMA.

kernel(**inputs) takes FULL inputs, shards over 8 cores, returns FULL output.
"""

import os
import numpy as np

B, N, P = 8, 1024, 128
HID, HEADS, NSTACK = 16, 8, 4
NT = N // P          # 8 row-tiles
JBLK = 8             # j rows per MLP block
NJB = N // JBLK      # 128 j-blocks
IC = 512             # i-chunk (matmul free dim)
NIC = N // IC        # 2

_CACHE = {}
LAST_RESULTS = None


def _emit(nc, tc, ctx, mm_dt):
    import concourse.bass as bass
    from concourse import mybir
    from concourse.masks import make_identity

    f32 = mybir.dt.float32

    adj = nc.declare_dram_parameter("adj", [N, N], f32, isOutput=False)
    # host-prepared block-diagonal weight layouts (see kernel())
    w1blk_d = nc.declare_dram_parameter("w1blk", [3 * JBLK, P], mm_dt, isOutput=False)
    w1diag_d = nc.declare_dram_parameter("w1diag", [JBLK, P], mm_dt, isOutput=False)
    w2blk_d = nc.declare_dram_parameter("w2blk", [P, JBLK * HEADS], mm_dt, isOutput=False)
    b1rep_d = nc.declare_dram_parameter("b1rep", [P, 1], f32, isOutput=False)
    i8_d = nc.declare_dram_parameter("i8", [JBLK, JBLK], mm_dt, isOutput=False)
    idn_d = nc.declare_dram_parameter("idn", [P, P], mm_dt, isOutput=False)
    idn32_d = nc.declare_dram_parameter("idn32", [P, P], f32, isOutput=False)
    out = nc.declare_dram_parameter("out", [N, N, HEADS], f32, isOutput=True)

    big = ctx.enter_context(tc.tile_pool(name="big", bufs=1))
    small = ctx.enter_context(tc.tile_pool(name="small", bufs=1))
    tpool = ctx.enter_context(tc.tile_pool(name="tpool", bufs=12))
    rpool = ctx.enter_context(tc.tile_pool(name="rpool", bufs=3))
    otpool = ctx.enter_context(tc.tile_pool(name="otpool", bufs=3))
    pt_ps = ctx.enter_context(tc.tile_pool(name="pt_ps", bufs=2, space="PSUM"))
    mm_ps = ctx.enter_context(tc.tile_pool(name="mm_ps", bufs=2, space="PSUM"))
    h_ps = ctx.enter_context(tc.tile_pool(name="h_ps", bufs=2, space="PSUM"))
    o_ps = ctx.enter_context(tc.tile_pool(name="o_ps", bufs=1, space="PSUM"))

    # persistent matrices (matmul-operand dtype), stored as [128, NT*1024]:
    # row-tile t at free cols [1024t, 1024t+1024), partition p = row 128t+p
    Af = big.tile([P, NT * N], f32, tag="Af")
    Qf = big.tile([P, NT * N], mm_dt, tag="Qf")
    Q2f = big.tile([P, NT * N], mm_dt, tag="Q2f")
    Q4f = big.tile([P, NT * N], mm_dt, tag="Q4f")
    invrep = big.tile([P, N], f32, tag="invrep")
    # manual ring buffer for the MLP's interleaved [j,s] rhs rows (NSEG slots)
    NSEG = 4
    ilbig = big.tile([3 * JBLK, NSEG * N], mm_dt, tag="ilbig")

    # ---- constants / weights (host-prepared; one DMA each) -----------------
    idn32 = small.tile([P, P], f32, tag="idn32")
    nc.gpsimd.dma_start(idn32[:], idn32_d[:])
    if mm_dt == f32:
        idn = idn32
    else:
        idn = small.tile([P, P], mm_dt, tag="idn")
        nc.gpsimd.dma_start(idn[:], idn_d[:])
    i8 = small.tile([JBLK, JBLK], mm_dt, tag="i8")
    nc.gpsimd.dma_start(i8[:], i8_d[:])
    ones1 = small.tile([1, P], f32, tag="ones1")
    nc.vector.memset(ones1[:], 1.0)
    w1blk = small.tile([3 * JBLK, P], mm_dt, tag="w1blk")
    nc.gpsimd.dma_start(w1blk[:], w1blk_d[:])
    w1diag = small.tile([JBLK, P], mm_dt, tag="w1diag")
    nc.gpsimd.dma_start(w1diag[:], w1diag_d[:])
    w2blk = small.tile([P, JBLK * HEADS], mm_dt, tag="w2blk")
    nc.gpsimd.dma_start(w2blk[:], w2blk_d[:])
    b1rep = small.tile([P, 1], f32, tag="b1rep")
    nc.gpsimd.dma_start(b1rep[:], b1rep_d[:])

    # ---- phase 1: deg -> invdeg (col-replicated) -> Q = A * invrep ----------
    invcol = small.tile([P, NT], f32, tag="invcol")
    for t in range(NT):
        nc.gpsimd.dma_start(
            Af[:, N * t:N * (t + 1)], adj[P * t:P * (t + 1), :]
        )
        deg = small.tile([P, 1], f32, tag=f"deg{t}")
        nc.vector.tensor_reduce(
            deg[:], Af[:, N * t:N * (t + 1)],
            axis=mybir.AxisListType.X, op=mybir.AluOpType.add,
        )
        degc = small.tile([P, 1], f32, tag=f"degc{t}")
        nc.vector.tensor_scalar_max(degc[:], deg[:], 1.0)
        nc.vector.reciprocal(invcol[:, t:t + 1], degc[:])

    invrow = small.tile([1, N], f32, tag="invrow")
    for t in range(NT):
        ptp = pt_ps.tile([P, P], f32, tag="pt")
        nc.tensor.transpose(ptp[0:1, :], invcol[:, t:t + 1], idn32[:])
        nc.scalar.copy(invrow[0:1, P * t:P * (t + 1)], ptp[0:1, :])
    for half in range(2):
        pb = mm_ps.tile([P, IC], f32, tag="mm")
        for k in range(4):
            c = 4 * half + k
            nc.tensor.matmul(
                pb[:, P * k:P * (k + 1)], ones1[:], invrow[0:1, P * c:P * (c + 1)],
                start=True, stop=True,
            )
        nc.scalar.copy(invrep[:, IC * half:IC * (half + 1)], pb[:])

    for t in range(NT):
        nc.vector.tensor_mul(
            Qf[:, N * t:N * (t + 1)], Af[:, N * t:N * (t + 1)], invrep[:]
        )

    # ---- phases 2+4: X2 = X @ X (lhsT tiles made on the fly by PE transpose)
    def square(src, dst):
        for al in range(NT):
            ts = []
            for g in range(NT):
                pp = pt_ps.tile([P, P], mm_dt, tag="pt")
                nc.tensor.transpose(
                    pp[:], src[:, N * al + P * g:N * al + P * (g + 1)], idn[:]
                )
                tg = tpool.tile([P, P], mm_dt, tag="T")
                nc.scalar.copy(tg[:], pp[:])
                ts.append(tg)
            for be in range(NIC):
                mm = mm_ps.tile([P, IC], f32, tag="mm")
                for g in range(NT):
                    nc.tensor.matmul(
                        mm[:], ts[g][:], src[:, N * g + IC * be:N * g + IC * (be + 1)],
                        start=(g == 0), stop=(g == NT - 1),
                    )
                nc.scalar.copy(dst[:, N * al + IC * be:N * al + IC * (be + 1)], mm[:])

    square(Qf, Q2f)
    square(Q2f, Q4f)

    # ---- phase 5: edge MLP + output transpose -------------------------------
    relu = mybir.ActivationFunctionType.Relu
    for pi in range(NJB // 2):
        po = [
            o_ps.tile([P, IC], f32, tag=f"O{ic}", name=f"po{ic}")
            for ic in range(NIC)
        ]
        for sub in range(2):
            jb = 2 * pi + sub
            trow = jb // (P // JBLK)
            prow = JBLK * (jb % (P // JBLK))
            seg = N * (jb % NSEG)
            for s, srcf in enumerate((Qf, Q2f, Q4f)):
                nc.gpsimd.dma_start(
                    ilbig[JBLK * s:JBLK * (s + 1), seg:seg + N],
                    srcf[prow:prow + JBLK, N * trow:N * (trow + 1)],
                )
            for ic in range(NIC):
                h = h_ps.tile([P, IC], f32, tag="H")
                nc.tensor.matmul(
                    h[:], w1blk[:], ilbig[:, seg + IC * ic:seg + IC * (ic + 1)],
                    start=True, stop=True,
                )
                if (JBLK * jb) // IC == ic:
                    off = JBLK * jb - IC * ic
                    nc.tensor.matmul(
                        h[:, off:off + JBLK], w1diag[:], i8[:],
                        start=False, stop=True, skip_group_check=True,
                    )
                rt = rpool.tile([P, IC], mm_dt, tag="R")
                nc.scalar.activation(rt[:], h[:], relu, bias=b1rep[:], scale=1.0)
                nc.tensor.matmul(
                    po[ic][64 * sub:64 * (sub + 1), :], w2blk[:], rt[:],
                    start=True, stop=True,
                )
        for ic in range(NIC):
            ot = otpool.tile([P, IC], f32, tag="OT")
            nc.vector.transpose(ot[:], po[ic][:])
            for g in range(4):
                dst = out[
                    IC * ic:IC * (ic + 1), 16 * pi + 4 * g:16 * pi + 4 * (g + 1), :
                ].rearrange("(f p) jl o -> p f (jl o)", p=32)
                src = ot[32 * g:32 * (g + 1), :].rearrange(
                    "p (f q) -> p f q", q=32
                )
                nc.gpsimd.dma_start(dst, src)


def _build(mm_dtype_name="float16"):
    key = mm_dtype_name
    if key in _CACHE:
        return _CACHE[key]
    from contextlib import ExitStack
    import concourse.tile as tile
    from concourse import bacc, mybir

    nc = bacc.Bacc()
    with tile.TileContext(nc) as tc:
        with ExitStack() as ctx:
            _emit(nc, tc, ctx, getattr(mybir.dt, mm_dtype_name))
    nc.compile()
    _CACHE[key] = nc
    return nc


def _install_ntff_shim():
    """The agent image's antenv lacks axon_hooks; provide it and register the
    ctypes NTFF hook so run_bass_kernel_spmd(trace=True) can profile."""
    import sys
    import types

    if "antenv.axon_hooks" in sys.modules:
        return
    mod = types.ModuleType("antenv.axon_hooks")
    mod._hook = None
    mod.set_axon_ntff_profile_hook = lambda h: setattr(mod, "_hook", h)
    mod.get_axon_ntff_profile_hook = lambda: mod._hook
    sys.modules["antenv.axon_hooks"] = mod
    try:
        from trn_agent_boot.trn_boot import _ntff_profile_via_ctypes

        mod._hook = _ntff_profile_via_ctypes("/opt/axon/libaxon_pjrt.so")
    except Exception as e:  # degrade to no-trace
        print(f"ntff shim install failed: {e}")


def kernel(adj, mask, w1, b1, w2, b2):
    from concourse.bass_utils import run_bass_kernel_spmd

    global LAST_RESULTS
    adj = np.ascontiguousarray(np.asarray(adj, dtype=np.float32))
    mask = np.asarray(mask)
    w1 = np.ascontiguousarray(np.asarray(w1, dtype=np.float32))
    b1 = np.ascontiguousarray(np.asarray(b1, dtype=np.float32))
    w2 = np.ascontiguousarray(np.asarray(w2, dtype=np.float32))
    b2 = np.asarray(b2, dtype=np.float32)
    assert adj.shape == (B, N, N), adj.shape

    m = mask.astype(np.float32)
    general_mask = not np.all(m == 1.0)
    if general_mask:
        pair = m[:, :, None] * m[:, None, :]
        adj = np.ascontiguousarray(adj * pair)

    trace = bool(int(os.environ.get("KERNEL_TRACE", "0")))
    if trace:
        _install_ntff_shim()
    mmname = os.environ.get("KERNEL_MM_DT", "float16")
    nc = _build(mmname)

    from concourse import mybir

    np_mm = mybir.dt.np(getattr(mybir.dt, mmname))
    w1blk_np = np.zeros((3 * JBLK, P), np.float32)
    w1diag_np = np.zeros((JBLK, P), np.float32)
    w2blk_np = np.zeros((P, JBLK * HEADS), np.float32)
    for j in range(JBLK):
        for s in range(3):
            w1blk_np[JBLK * s + j, HID * j:HID * (j + 1)] = w1[s + 1]
        w1diag_np[j, HID * j:HID * (j + 1)] = w1[0]
        w2blk_np[HID * j:HID * (j + 1), HEADS * j:HEADS * (j + 1)] = w2
    shared = {
        "w1blk": w1blk_np.astype(np_mm),
        "w1diag": w1diag_np.astype(np_mm),
        "w2blk": w2blk_np.astype(np_mm),
        "b1rep": np.ascontiguousarray(np.tile(b1, JBLK).astype(np.float32)[:, None]),
        "i8": np.eye(JBLK, dtype=np_mm),
        "idn": np.eye(P, dtype=np_mm),
        "idn32": np.eye(P, dtype=np.float32),
    }
    in_maps = [{"adj": adj[c], **shared} for c in range(B)]
    res = run_bass_kernel_spmd(nc, in_maps, list(range(B)), trace=trace)
    LAST_RESULTS = res
    outp = np.stack([res.results[c]["out"] for c in range(B)], axis=0)

    if np.any(b2 != 0.0):
        outp = outp + b2
    if general_mask:
        outp = outp * pair[..., None]
    return np.ascontiguousarray(outp.astype(np.float32))


# revision 30
# speedup vs baseline: 1.0615x; 1.0615x over previous
"""Trainium2 Bass kernel for nn_Diffuser_78331613544465.

Math (per graph b of B=8, N=1024):
    A   = adj (mask is all-ones in the graded setup; general mask handled host-side)
    P   = A / max(rowsum(A), 1)
    out[i,j,:] = relu([I, P, P2, P4][i,j,:] @ w1 + b1) @ w2 + b2   (P2=P@P, P4=P2@P2)

Device strategy: data-parallel over B — one graph per NeuronCore (8 cores).
All on-chip work happens in the TRANSPOSED domain (Q = P^T), because
  * Q = A * (1/deg) is a column-scale of the symmetric adj,
  * Q2 = Q@Q = P2^T, Q4 = Q2@Q2 = P4^T chain natively,
  * the edge-MLP then runs with j on partitions / i on the moving dim, which
    makes layer-1 a K=24 block-diagonal matmul over interleaved (j,s) rows,
    layer-2 a K=128 block-diagonal matmul, and the final 32x32 DVE stream
    transpose emits [i-partition, (j,o)-contiguous] tiles for 128B-chunk DMA.

kernel(**inputs) takes FULL inputs, shards over 8 cores, returns FULL output.
"""

import os
import numpy as np

B, N, P = 8, 1024, 128
HID, HEADS, NSTACK = 16, 8, 4
NT = N // P          # 8 row-tiles
JBLK = 8             # j rows per MLP block
NJB = N // JBLK      # 128 j-blocks
IC = 512             # i-chunk (matmul free dim)
NIC = N // IC        # 2

_CACHE = {}
LAST_RESULTS = None


def _emit(nc, tc, ctx, mm_dt):
    import concourse.bass as bass
    from concourse import mybir
    from concourse.masks import make_identity

    f32 = mybir.dt.float32

    adj = nc.declare_dram_parameter("adj", [N, N], f32, isOutput=False)
    # host-prepared block-diagonal weight layouts (see kernel())
    w1blk_d = nc.declare_dram_parameter("w1blk", [3 * JBLK, P], mm_dt, isOutput=False)
    w1diag_d = nc.declare_dram_parameter("w1diag", [JBLK, P], mm_dt, isOutput=False)
    w2blk_d = nc.declare_dram_parameter("w2blk", [P, JBLK * HEADS], mm_dt, isOutput=False)
    b1rep_d = nc.declare_dram_parameter("b1rep", [P, 1], f32, isOutput=False)
    i8_d = nc.declare_dram_parameter("i8", [JBLK, JBLK], mm_dt, isOutput=False)
    idn_d = nc.declare_dram_parameter("idn", [P, P], mm_dt, isOutput=False)
    idn32_d = nc.declare_dram_parameter("idn32", [P, P], f32, isOutput=False)
    out = nc.declare_dram_parameter("out", [N, N, HEADS], f32, isOutput=True)

    big = ctx.enter_context(tc.tile_pool(name="big", bufs=1))
    small = ctx.enter_context(tc.tile_pool(name="small", bufs=1))
    tpool = ctx.enter_context(tc.tile_pool(name="tpool", bufs=12))
    rpool = ctx.enter_context(tc.tile_pool(name="rpool", bufs=3))
    otpool = ctx.enter_context(tc.tile_pool(name="otpool", bufs=3))
    pt_ps = ctx.enter_context(tc.tile_pool(name="pt_ps", bufs=2, space="PSUM"))
    mm_ps = ctx.enter_context(tc.tile_pool(name="mm_ps", bufs=2, space="PSUM"))
    h_ps = ctx.enter_context(tc.tile_pool(name="h_ps", bufs=2, space="PSUM"))
    o_ps = ctx.enter_context(tc.tile_pool(name="o_ps", bufs=1, space="PSUM"))

    # persistent matrices (matmul-operand dtype), stored as [128, NT*1024]:
    # row-tile t at free cols [1024t, 1024t+1024), partition p = row 128t+p
    Af = big.tile([P, NT * N], f32, tag="Af")
    Qf = big.tile([P, NT * N], mm_dt, tag="Qf")
    Q2f = big.tile([P, NT * N], mm_dt, tag="Q2f")
    Q4f = big.tile([P, NT * N], mm_dt, tag="Q4f")
    invrep = big.tile([P, N], f32, tag="invrep")
    # manual ring buffer for the MLP's interleaved [j,s] rhs rows (NSEG slots)
    NSEG = 4
    ilbig = big.tile([3 * JBLK, NSEG * N], mm_dt, tag="ilbig")

    # ---- constants / weights (host-prepared; one DMA each) -----------------
    idn32 = small.tile([P, P], f32, tag="idn32")
    nc.sync.dma_start(idn32[:], idn32_d[:])
    if mm_dt == f32:
        idn = idn32
    else:
        idn = small.tile([P, P], mm_dt, tag="idn")
        nc.sync.dma_start(idn[:], idn_d[:])
    i8 = small.tile([JBLK, JBLK], mm_dt, tag="i8")
    nc.sync.dma_start(i8[:], i8_d[:])
    ones1 = small.tile([1, P], f32, tag="ones1")
    nc.vector.memset(ones1[:], 1.0)
    w1blk = small.tile([3 * JBLK, P], mm_dt, tag="w1blk")
    nc.sync.dma_start(w1blk[:], w1blk_d[:])
    w1diag = small.tile([JBLK, P], mm_dt, tag="w1diag")
    nc.sync.dma_start(w1diag[:], w1diag_d[:])
    w2blk = small.tile([P, JBLK * HEADS], mm_dt, tag="w2blk")
    nc.sync.dma_start(w2blk[:], w2blk_d[:])
    b1rep = small.tile([P, 1], f32, tag="b1rep")
    nc.sync.dma_start(b1rep[:], b1rep_d[:])

    # ---- phase 1: deg -> invdeg (col-replicated) -> Q = A * invrep ----------
    invcol = small.tile([P, NT], f32, tag="invcol")
    for t in range(NT):
        nc.sync.dma_start(
            Af[:, N * t:N * (t + 1)], adj[P * t:P * (t + 1), :]
        )
        deg = small.tile([P, 1], f32, tag=f"deg{t}")
        nc.vector.tensor_reduce(
            deg[:], Af[:, N * t:N * (t + 1)],
            axis=mybir.AxisListType.X, op=mybir.AluOpType.add,
        )
        degc = small.tile([P, 1], f32, tag=f"degc{t}")
        nc.vector.tensor_scalar_max(degc[:], deg[:], 1.0)
        nc.vector.reciprocal(invcol[:, t:t + 1], degc[:])

    invrow = small.tile([1, N], f32, tag="invrow")
    for t in range(NT):
        ptp = pt_ps.tile([P, P], f32, tag="pt")
        nc.tensor.transpose(ptp[0:1, :], invcol[:, t:t + 1], idn32[:])
        nc.scalar.copy(invrow[0:1, P * t:P * (t + 1)], ptp[0:1, :])
    for half in range(2):
        pb = mm_ps.tile([P, IC], f32, tag="mm")
        for k in range(4):
            c = 4 * half + k
            nc.tensor.matmul(
                pb[:, P * k:P * (k + 1)], ones1[:], invrow[0:1, P * c:P * (c + 1)],
                start=True, stop=True,
            )
        nc.scalar.copy(invrep[:, IC * half:IC * (half + 1)], pb[:])

    for t in range(NT):
        nc.vector.tensor_mul(
            Qf[:, N * t:N * (t + 1)], Af[:, N * t:N * (t + 1)], invrep[:]
        )

    # ---- phases 2+4: X2 = X @ X (lhsT tiles made on the fly by PE transpose)
    def square(src, dst):
        for al in range(NT):
            ts = []
            for g in range(NT):
                pp = pt_ps.tile([P, P], mm_dt, tag="pt")
                nc.tensor.transpose(
                    pp[:], src[:, N * al + P * g:N * al + P * (g + 1)], idn[:]
                )
                tg = tpool.tile([P, P], mm_dt, tag="T")
                nc.scalar.copy(tg[:], pp[:])
                ts.append(tg)
            for be in range(NIC):
                mm = mm_ps.tile([P, IC], f32, tag="mm")
                for g in range(NT):
                    nc.tensor.matmul(
                        mm[:], ts[g][:], src[:, N * g + IC * be:N * g + IC * (be + 1)],
                        start=(g == 0), stop=(g == NT - 1),
                    )
                nc.scalar.copy(dst[:, N * al + IC * be:N * al + IC * (be + 1)], mm[:])

    square(Qf, Q2f)
    square(Q2f, Q4f)

    # ---- phase 5: edge MLP + output transpose -------------------------------
    relu = mybir.ActivationFunctionType.Relu
    for pi in range(NJB // 2):
        po = [
            o_ps.tile([P, IC], f32, tag=f"O{ic}", name=f"po{ic}")
            for ic in range(NIC)
        ]
        for sub in range(2):
            jb = 2 * pi + sub
            trow = jb // (P // JBLK)
            prow = JBLK * (jb % (P // JBLK))
            seg = N * (jb % NSEG)
            for s, srcf in enumerate((Qf, Q2f, Q4f)):
                nc.sync.dma_start(
                    ilbig[JBLK * s:JBLK * (s + 1), seg:seg + N],
                    srcf[prow:prow + JBLK, N * trow:N * (trow + 1)],
                )
            for ic in range(NIC):
                h = h_ps.tile([P, IC], f32, tag="H")
                nc.tensor.matmul(
                    h[:], w1blk[:], ilbig[:, seg + IC * ic:seg + IC * (ic + 1)],
                    start=True, stop=True,
                )
                if (JBLK * jb) // IC == ic:
                    off = JBLK * jb - IC * ic
                    nc.tensor.matmul(
                        h[:, off:off + JBLK], w1diag[:], i8[:],
                        start=False, stop=True, skip_group_check=True,
                    )
                rt = rpool.tile([P, IC], mm_dt, tag="R")
                nc.scalar.activation(rt[:], h[:], relu, bias=b1rep[:], scale=1.0)
                nc.tensor.matmul(
                    po[ic][64 * sub:64 * (sub + 1), :], w2blk[:], rt[:],
                    start=True, stop=True,
                )
        for ic in range(NIC):
            ot = otpool.tile([P, IC], f32, tag="OT")
            nc.vector.transpose(ot[:], po[ic][:])
            for g in range(4):
                dst = out[
                    IC * ic:IC * (ic + 1), 16 * pi + 4 * g:16 * pi + 4 * (g + 1), :
                ].rearrange("(f p) jl o -> p f (jl o)", p=32)
                src = ot[32 * g:32 * (g + 1), :].rearrange(
                    "p (f q) -> p f q", q=32
                )
                nc.sync.dma_start(dst, src)


def _build(mm_dtype_name="float16"):
    key = mm_dtype_name
    if key in _CACHE:
        return _CACHE[key]
    from contextlib import ExitStack
    import concourse.tile as tile
    from concourse import bacc, mybir

    nc = bacc.Bacc()
    with tile.TileContext(nc) as tc:
        with ExitStack() as ctx:
            _emit(nc, tc, ctx, getattr(mybir.dt, mm_dtype_name))
    nc.compile()
    _CACHE[key] = nc
    return nc


def _install_ntff_shim():
    """The agent image's antenv lacks axon_hooks; provide it and register the
    ctypes NTFF hook so run_bass_kernel_spmd(trace=True) can profile."""
    import sys
    import types

    if "antenv.axon_hooks" in sys.modules:
        return
    mod = types.ModuleType("antenv.axon_hooks")
    mod._hook = None
    mod.set_axon_ntff_profile_hook = lambda h: setattr(mod, "_hook", h)
    mod.get_axon_ntff_profile_hook = lambda: mod._hook
    sys.modules["antenv.axon_hooks"] = mod
    try:
        from trn_agent_boot.trn_boot import _ntff_profile_via_ctypes

        mod._hook = _ntff_profile_via_ctypes("/opt/axon/libaxon_pjrt.so")
    except Exception as e:  # degrade to no-trace
        print(f"ntff shim install failed: {e}")


def kernel(adj, mask, w1, b1, w2, b2):
    from concourse.bass_utils import run_bass_kernel_spmd

    global LAST_RESULTS
    adj = np.ascontiguousarray(np.asarray(adj, dtype=np.float32))
    mask = np.asarray(mask)
    w1 = np.ascontiguousarray(np.asarray(w1, dtype=np.float32))
    b1 = np.ascontiguousarray(np.asarray(b1, dtype=np.float32))
    w2 = np.ascontiguousarray(np.asarray(w2, dtype=np.float32))
    b2 = np.asarray(b2, dtype=np.float32)
    assert adj.shape == (B, N, N), adj.shape

    m = mask.astype(np.float32)
    general_mask = not np.all(m == 1.0)
    if general_mask:
        pair = m[:, :, None] * m[:, None, :]
        adj = np.ascontiguousarray(adj * pair)

    trace = bool(int(os.environ.get("KERNEL_TRACE", "0")))
    if trace:
        _install_ntff_shim()
    mmname = os.environ.get("KERNEL_MM_DT", "float16")
    nc = _build(mmname)

    from concourse import mybir

    np_mm = mybir.dt.np(getattr(mybir.dt, mmname))
    w1blk_np = np.zeros((3 * JBLK, P), np.float32)
    w1diag_np = np.zeros((JBLK, P), np.float32)
    w2blk_np = np.zeros((P, JBLK * HEADS), np.float32)
    for j in range(JBLK):
        for s in range(3):
            w1blk_np[JBLK * s + j, HID * j:HID * (j + 1)] = w1[s + 1]
        w1diag_np[j, HID * j:HID * (j + 1)] = w1[0]
        w2blk_np[HID * j:HID * (j + 1), HEADS * j:HEADS * (j + 1)] = w2
    shared = {
        "w1blk": w1blk_np.astype(np_mm),
        "w1diag": w1diag_np.astype(np_mm),
        "w2blk": w2blk_np.astype(np_mm),
        "b1rep": np.ascontiguousarray(np.tile(b1, JBLK).astype(np.float32)[:, None]),
        "i8": np.eye(JBLK, dtype=np_mm),
        "idn": np.eye(P, dtype=np_mm),
        "idn32": np.eye(P, dtype=np.float32),
    }
    in_maps = [{"adj": adj[c], **shared} for c in range(B)]
    res = run_bass_kernel_spmd(nc, in_maps, list(range(B)), trace=trace)
    LAST_RESULTS = res
    outp = np.stack([res.results[c]["out"] for c in range(B)], axis=0)

    if np.any(b2 != 0.0):
        outp = outp + b2
    if general_mask:
        outp = outp * pair[..., None]
    return np.ascontiguousarray(outp.astype(np.float32))


# revision 38
# speedup vs baseline: 1.8475x; 1.7405x over previous
"""Trainium2 Bass kernel for nn_Diffuser_78331613544465.

Math (per graph b of B=8, N=1024):
    A   = adj (mask is all-ones in the graded setup; general mask handled host-side)
    P   = A / max(rowsum(A), 1)
    out[i,j,:] = relu([I, P, P2, P4][i,j,:] @ w1 + b1) @ w2 + b2   (P2=P@P, P4=P2@P2)

Device strategy: data-parallel over B — one graph per NeuronCore (8 cores).
All on-chip work happens in the TRANSPOSED domain (Q = P^T), because
  * Q = A * (1/deg) is a column-scale of the symmetric adj,
  * Q2 = Q@Q = P2^T, Q4 = Q2@Q2 = P4^T chain natively,
  * the edge-MLP then runs with j on partitions / i on the moving dim, which
    makes layer-1 a K=24 block-diagonal matmul over interleaved (j,s) rows,
    layer-2 a K=128 block-diagonal matmul, and the final 32x32 DVE stream
    transpose emits [i-partition, (j,o)-contiguous] tiles for 128B-chunk DMA.

kernel(**inputs) takes FULL inputs, shards over 8 cores, returns FULL output.
"""

import os
import numpy as np

B, N, P = 8, 1024, 128
HID, HEADS, NSTACK = 16, 8, 4
NT = N // P          # 8 row-tiles
JBLK = 8             # j rows per MLP block
NJB = N // JBLK      # 128 j-blocks
IC = 512             # i-chunk (matmul free dim)
NIC = N // IC        # 2

_CACHE = {}
LAST_RESULTS = None


def _emit(nc, tc, ctx, mm_dt):
    import concourse.bass as bass
    from concourse import mybir
    from concourse.masks import make_identity

    f32 = mybir.dt.float32

    adj = nc.declare_dram_parameter("adj", [N, N], f32, isOutput=False)
    # host-prepared block-diagonal weight layouts (see kernel())
    w1blk_d = nc.declare_dram_parameter("w1blk", [3 * JBLK, P], mm_dt, isOutput=False)
    w1diag_d = nc.declare_dram_parameter("w1diag", [JBLK, P], mm_dt, isOutput=False)
    w2blk_d = nc.declare_dram_parameter("w2blk", [P, JBLK * HEADS], mm_dt, isOutput=False)
    b1rep_d = nc.declare_dram_parameter("b1rep", [P, 1], f32, isOutput=False)
    i8_d = nc.declare_dram_parameter("i8", [JBLK, JBLK], mm_dt, isOutput=False)
    idn_d = nc.declare_dram_parameter("idn", [P, P], mm_dt, isOutput=False)
    idn32_d = nc.declare_dram_parameter("idn32", [P, P], f32, isOutput=False)
    out = nc.declare_dram_parameter("out", [N, N, HEADS], f32, isOutput=True)

    from contextlib import ExitStack

    big = ctx.enter_context(tc.tile_pool(name="big", bufs=1))
    small = ctx.enter_context(tc.tile_pool(name="small", bufs=1))
    tpool = ctx.enter_context(tc.tile_pool(name="tpool", bufs=12))
    rpool = ctx.enter_context(tc.tile_pool(name="rpool", bufs=3))
    otpool = ctx.enter_context(tc.tile_pool(name="otpool", bufs=3))
    ph14 = ExitStack()
    pt_ps = ph14.enter_context(tc.tile_pool(name="pt_ps", bufs=2, space="PSUM"))
    mm_ps = ph14.enter_context(tc.tile_pool(name="mm_ps", bufs=2, space="PSUM"))

    # persistent matrices (matmul-operand dtype), stored as [128, NT*1024]:
    # row-tile t at free cols [1024t, 1024t+1024), partition p = row 128t+p
    Af = big.tile([P, NT * N], f32, tag="Af")
    Qf = big.tile([P, NT * N], mm_dt, tag="Qf")
    Q2f = big.tile([P, NT * N], mm_dt, tag="Q2f")
    Q4f = big.tile([P, NT * N], mm_dt, tag="Q4f")
    invrep = big.tile([P, N], f32, tag="invrep")
    # manual ring buffer for the MLP's interleaved [j,s] rhs rows (NSEG slots,
    # filled 4 j-blocks per DMA, double-ring)
    NSEG = 8
    ilbig = big.tile([3 * JBLK, NSEG * N], mm_dt, tag="ilbig")
    # DRAM spills of Q/Q2/Q4: the IL batch-loads need APs that hop rows
    # freely, which only DRAM-side APs allow
    dram = ctx.enter_context(tc.tile_pool(name="dram", bufs=1, space="DRAM"))
    Qd = dram.tile([N, N], mm_dt, tag="Qd")
    Q2d = dram.tile([N, N], mm_dt, tag="Q2d")
    Q4d = dram.tile([N, N], mm_dt, tag="Q4d")

    def spill(srcf, dstd):
        nc.gpsimd.dma_start(
            dstd[:].rearrange("(t p) c -> p t c", p=P),
            srcf[:].rearrange("p (t c) -> p t c", c=N),
        )

    # ---- constants / weights (host-prepared; one DMA each) -----------------
    idn32 = small.tile([P, P], f32, tag="idn32")
    nc.gpsimd.dma_start(idn32[:], idn32_d[:])
    if mm_dt == f32:
        idn = idn32
    else:
        idn = small.tile([P, P], mm_dt, tag="idn")
        nc.gpsimd.dma_start(idn[:], idn_d[:])
    i8 = small.tile([JBLK, JBLK], mm_dt, tag="i8")
    nc.gpsimd.dma_start(i8[:], i8_d[:])
    ones1 = small.tile([1, P], f32, tag="ones1")
    nc.vector.memset(ones1[:], 1.0)
    w1blk = small.tile([3 * JBLK, P], mm_dt, tag="w1blk")
    nc.gpsimd.dma_start(w1blk[:], w1blk_d[:])
    w1diag = small.tile([JBLK, P], mm_dt, tag="w1diag")
    nc.gpsimd.dma_start(w1diag[:], w1diag_d[:])
    w2blk = small.tile([P, JBLK * HEADS], mm_dt, tag="w2blk")
    nc.gpsimd.dma_start(w2blk[:], w2blk_d[:])
    b1rep = small.tile([P, 1], f32, tag="b1rep")
    nc.gpsimd.dma_start(b1rep[:], b1rep_d[:])

    # ---- phase 1: deg -> invdeg (col-replicated) -> Q = A * invrep ----------
    invcol = small.tile([P, NT], f32, tag="invcol")
    for t in range(NT):
        nc.gpsimd.dma_start(
            Af[:, N * t:N * (t + 1)], adj[P * t:P * (t + 1), :]
        )
        deg = small.tile([P, 1], f32, tag=f"deg{t}")
        nc.vector.tensor_reduce(
            deg[:], Af[:, N * t:N * (t + 1)],
            axis=mybir.AxisListType.X, op=mybir.AluOpType.add,
        )
        degc = small.tile([P, 1], f32, tag=f"degc{t}")
        nc.vector.tensor_scalar_max(degc[:], deg[:], 1.0)
        nc.vector.reciprocal(invcol[:, t:t + 1], degc[:])

    invrow = small.tile([1, N], f32, tag="invrow")
    for t in range(NT):
        ptp = pt_ps.tile([P, P], f32, tag="pt")
        nc.tensor.transpose(ptp[0:1, :], invcol[:, t:t + 1], idn32[:])
        nc.scalar.copy(invrow[0:1, P * t:P * (t + 1)], ptp[0:1, :])
    for half in range(2):
        pb = mm_ps.tile([P, IC], f32, tag="mm")
        for k in range(4):
            c = 4 * half + k
            nc.tensor.matmul(
                pb[:, P * k:P * (k + 1)], ones1[:], invrow[0:1, P * c:P * (c + 1)],
                start=True, stop=True,
            )
        nc.scalar.copy(invrep[:, IC * half:IC * (half + 1)], pb[:])

    for t in range(NT):
        nc.vector.tensor_mul(
            Qf[:, N * t:N * (t + 1)], Af[:, N * t:N * (t + 1)], invrep[:]
        )

    # ---- phases 2+4: X2 = X @ X (lhsT tiles made on the fly by PE transpose)
    def square(src, dst):
        for al in range(NT):
            ts = []
            for g in range(NT):
                pp = pt_ps.tile([P, P], mm_dt, tag="pt")
                nc.tensor.transpose(
                    pp[:], src[:, N * al + P * g:N * al + P * (g + 1)], idn[:]
                )
                tg = tpool.tile([P, P], mm_dt, tag="T")
                nc.scalar.copy(tg[:], pp[:])
                ts.append(tg)
            for be in range(NIC):
                mm = mm_ps.tile([P, IC], f32, tag="mm")
                for g in range(NT):
                    nc.tensor.matmul(
                        mm[:], ts[g][:], src[:, N * g + IC * be:N * g + IC * (be + 1)],
                        start=(g == 0), stop=(g == NT - 1),
                    )
                nc.scalar.copy(dst[:, N * al + IC * be:N * al + IC * (be + 1)], mm[:])

    spill(Qf, Qd)
    square(Qf, Q2f)
    spill(Q2f, Q2d)
    square(Q2f, Q4f)
    spill(Q4f, Q4d)
    ph14.close()  # free pt/mm PSUM banks for the MLP pools

    h_ps = ctx.enter_context(tc.tile_pool(name="h_ps", bufs=2, space="PSUM"))
    o_ps = ctx.enter_context(tc.tile_pool(name="o_ps", bufs=2, space="PSUM"))

    # ---- phase 5: edge MLP + output transpose -------------------------------
    relu = mybir.ActivationFunctionType.Relu
    for pi in range(NJB // 2):
        po = o_ps.tile([P, N], f32, tag="O")   # [128, 1024]: (jb-pair, all i)
        for sub in range(2):
            jb = 2 * pi + sub
            trow = jb // (P // JBLK)
            prow = JBLK * (jb % (P // JBLK))
            seg = N * (jb % NSEG)
            if jb % 4 == 0:
                # fill 4 ring segments (4 j-blocks) per channel in one DMA:
                # dst [kk(8, partition), (jj c)(4096)]; src rows 8jb..8jb+32
                # of the DRAM spill, traversed kk-outer
                base = N * (jb % NSEG)
                for srcd in (Qd, Q2d, Q4d):
                    s = (Qd, Q2d, Q4d).index(srcd)
                    nc.gpsimd.dma_start(
                        ilbig[JBLK * s:JBLK * (s + 1), base:base + 4 * N],
                        srcd[JBLK * jb:JBLK * jb + 4 * JBLK, :].rearrange(
                            "(jj kk) c -> kk jj c", kk=JBLK
                        ),
                    )
            for ic in range(NIC):
                h = h_ps.tile([P, IC], f32, tag="H")
                nc.tensor.matmul(
                    h[:], w1blk[:], ilbig[:, seg + IC * ic:seg + IC * (ic + 1)],
                    start=True, stop=True,
                )
                if (JBLK * jb) // IC == ic:
                    off = JBLK * jb - IC * ic
                    nc.tensor.matmul(
                        h[:, off:off + JBLK], w1diag[:], i8[:],
                        start=False, stop=True, skip_group_check=True,
                    )
                rt = rpool.tile([P, IC], mm_dt, tag="R")
                nc.scalar.activation(rt[:], h[:], relu, bias=b1rep[:], scale=1.0)
                nc.tensor.matmul(
                    po[64 * sub:64 * (sub + 1), IC * ic:IC * (ic + 1)],
                    w2blk[:], rt[:], start=True, stop=True,
                )
        ot = otpool.tile([P, N], f32, tag="OT")
        nc.vector.transpose(ot[:], po[:])
        for g in range(4):
            dst = out[
                :, 16 * pi + 4 * g:16 * pi + 4 * (g + 1), :
            ].rearrange("(f p) jl o -> p f (jl o)", p=32)
            src = ot[32 * g:32 * (g + 1), :].rearrange("p (f q) -> p f q", q=32)
            nc.sync.dma_start(dst, src)


def _build(mm_dtype_name="float16"):
    key = mm_dtype_name
    if key in _CACHE:
        return _CACHE[key]
    from contextlib import ExitStack
    import concourse.tile as tile
    from concourse import bacc, mybir

    nc = bacc.Bacc()
    with tile.TileContext(nc) as tc:
        with ExitStack() as ctx:
            _emit(nc, tc, ctx, getattr(mybir.dt, mm_dtype_name))
    nc.compile()
    _CACHE[key] = nc
    return nc


def _install_ntff_shim():
    """The agent image's antenv lacks axon_hooks; provide it and register the
    ctypes NTFF hook so run_bass_kernel_spmd(trace=True) can profile."""
    import sys
    import types

    if "antenv.axon_hooks" in sys.modules:
        return
    mod = types.ModuleType("antenv.axon_hooks")
    mod._hook = None
    mod.set_axon_ntff_profile_hook = lambda h: setattr(mod, "_hook", h)
    mod.get_axon_ntff_profile_hook = lambda: mod._hook
    sys.modules["antenv.axon_hooks"] = mod
    try:
        from trn_agent_boot.trn_boot import _ntff_profile_via_ctypes

        mod._hook = _ntff_profile_via_ctypes("/opt/axon/libaxon_pjrt.so")
    except Exception as e:  # degrade to no-trace
        print(f"ntff shim install failed: {e}")


def kernel(adj, mask, w1, b1, w2, b2):
    from concourse.bass_utils import run_bass_kernel_spmd

    global LAST_RESULTS
    adj = np.ascontiguousarray(np.asarray(adj, dtype=np.float32))
    mask = np.asarray(mask)
    w1 = np.ascontiguousarray(np.asarray(w1, dtype=np.float32))
    b1 = np.ascontiguousarray(np.asarray(b1, dtype=np.float32))
    w2 = np.ascontiguousarray(np.asarray(w2, dtype=np.float32))
    b2 = np.asarray(b2, dtype=np.float32)
    assert adj.shape == (B, N, N), adj.shape

    m = mask.astype(np.float32)
    general_mask = not np.all(m == 1.0)
    if general_mask:
        pair = m[:, :, None] * m[:, None, :]
        adj = np.ascontiguousarray(adj * pair)

    trace = bool(int(os.environ.get("KERNEL_TRACE", "0")))
    if trace:
        _install_ntff_shim()
    mmname = os.environ.get("KERNEL_MM_DT", "float16")
    nc = _build(mmname)

    from concourse import mybir

    np_mm = mybir.dt.np(getattr(mybir.dt, mmname))
    w1blk_np = np.zeros((3 * JBLK, P), np.float32)
    w1diag_np = np.zeros((JBLK, P), np.float32)
    w2blk_np = np.zeros((P, JBLK * HEADS), np.float32)
    for j in range(JBLK):
        for s in range(3):
            w1blk_np[JBLK * s + j, HID * j:HID * (j + 1)] = w1[s + 1]
        w1diag_np[j, HID * j:HID * (j + 1)] = w1[0]
        w2blk_np[HID * j:HID * (j + 1), HEADS * j:HEADS * (j + 1)] = w2
    shared = {
        "w1blk": w1blk_np.astype(np_mm),
        "w1diag": w1diag_np.astype(np_mm),
        "w2blk": w2blk_np.astype(np_mm),
        "b1rep": np.ascontiguousarray(np.tile(b1, JBLK).astype(np.float32)[:, None]),
        "i8": np.eye(JBLK, dtype=np_mm),
        "idn": np.eye(P, dtype=np_mm),
        "idn32": np.eye(P, dtype=np.float32),
    }
    in_maps = [{"adj": adj[c], **shared} for c in range(B)]
    res = run_bass_kernel_spmd(nc, in_maps, list(range(B)), trace=trace)
    LAST_RESULTS = res
    outp = np.stack([res.results[c]["out"] for c in range(B)], axis=0)

    if np.any(b2 != 0.0):
        outp = outp + b2
    if general_mask:
        outp = outp * pair[..., None]
    return np.ascontiguousarray(outp.astype(np.float32))
